# revision 53
# baseline (speedup 1.0000x reference)
"""Bass/Trainium2 kernel for HCFC-GNN (3-layer GCN + hierarchy max-constraint).

v2 strategy (8 NeuronCores, SPMD), exploiting that aggregation commutes with
the dense transforms:
  - L1: aggregate the 16-wide table [dinv*x, dinv, 0, 0] (built locally on
    every core from the replicated x input -- no collective), via 32B-element
    dma_gathers (elem_size=16, elem_step=128 on a 256B-strided table) and a
    tree-add segment reduction over per-target slots; then
    h1 = relu(dinv * (z1 @ [W1^T; b1; 0])) using the aggregated dinv column
    as the bias multiplier.
  - L2: g2 = dinv*h1 as an fp8 table (rows padded to 256B stride), AllGather,
    128B-element gathers, one-hot S (DVE is_equal vs iota) and PE scatter
    matmuls S^T @ msg into PSUM (mixed bf16 x fp8); transform + bias via
    s_raw = z1[:,12] saved from L1.
  - L3: transform first (g3 = dinv*(h2 @ W3^T + b3), 13-wide), tiny AllGather,
    32B-element gathers + tree-add, sigmoid, hierarchy R-max.
  - Nodes are permuted per-core by (deg_lo, deg_hi) so tree slot counts per
    128-target block are tight; the host unpermutes the final output.
"""

import os
import numpy as np
import ml_dtypes

N = 50000
E = 1600000
C = 13
DIN = 12
H = 128
NCORES = 8
SH = N // NCORES          # 6250 nodes per shard
CH = 6272                 # chunk rows per core in tables (6250 + 22 zero pad)
BLK = (SH + 127) // 128   # 49 blocks per shard
LASTB = SH - (BLK - 1) * 128  # 106 rows in last block
HALF = 4 * CH             # 25088 rows per gather half (int16-safe)
ZROW = SH                 # zero-row index inside each half
PADCREL = 300.0           # colrel value that never matches iota 0..127
G1 = 12                    # tree group size (blocks)
G2 = 4                    # S-matmul group size (blocks)
NG1 = (BLK + G1 - 1) // G1
NG2 = (BLK + G2 - 1) // G2

bf16 = ml_dtypes.bfloat16
f8 = ml_dtypes.float8_e4m3

LAST_RESULTS = None


def _wrap16(idx_flat):
    """int64 slot-ordered idx array (len % 128 == 0) -> [128, len/16] int16
    wrapped layout (pos i -> partition i%16, col i//16; replicated x8)."""
    m = idx_flat.shape[0]
    g = idx_flat.reshape(m // 16, 16).T.astype(np.int16)
    return np.tile(g, (8, 1)).copy()


def _prep_edges(edge_index):
    row = np.concatenate([edge_index[0], np.arange(N, dtype=np.int32)])
    col = np.concatenate([edge_index[1], np.arange(N, dtype=np.int32)])
    deg = np.bincount(row, minlength=N).astype(np.float32)
    dinv = 1.0 / np.sqrt(deg)

    half_of = (row // SH) >= 4                      # source half (cores 4-7)
    dlo = np.bincount(col[~half_of], minlength=N)
    dhi = np.bincount(col[half_of], minlength=N)

    # per-core node permutation by (dlo, dhi)
    def _morton(a, b):
        out = np.zeros_like(a)
        for i in range(8):
            out |= ((a >> i) & 1) << (2 * i + 1)
            out |= ((b >> i) & 1) << (2 * i)
        return out

    perms = []
    gpos = np.empty(N, np.int64)                    # node -> global table row
    for k in range(NCORES):
        sl = slice(k * SH, (k + 1) * SH)
        # morton order over (dlo, dhi): blocks cover compact 2D degree
        # tiles, keeping per-block maxes (tree slot counts) tight
        p = np.argsort(_morton(dlo[sl].astype(np.int64),
                               dhi[sl].astype(np.int64)), kind="stable")
        perms.append(p)
        gpos[k * SH + p] = k * CH + np.arange(SH)

    grow = gpos[row]
    ehalf = (grow >= HALF).astype(np.int64)
    gloc = np.where(ehalf == 0, grow, grow - HALF)
    tcore = col // SH
    tpos = gpos[col] - tcore * CH                   # local permuted position
    tblk = tpos // 128
    trel = tpos % 128

    # ---- tree stream (A1/A3): per group of G1 blocks, [lo tiles | hi tiles]
    dlo_p = dlo[np.concatenate([k * SH + p for k, p in enumerate(perms)])]
    dhi_p = dhi[np.concatenate([k * SH + p for k, p in enumerate(perms)])]
    dlo_p = dlo_p.reshape(NCORES, SH)
    dhi_p = dhi_p.reshape(NCORES, SH)
    # per-block cross-core maxes
    blk_klo = np.ones(BLK, np.int64)
    blk_khi = np.ones(BLK, np.int64)
    for b in range(BLK):
        blk_klo[b] = max(1, int(dlo_p[:, b * 128:(b + 1) * 128].max()))
        blk_khi[b] = max(1, int(dhi_p[:, b * 128:(b + 1) * 128].max()))
    dlo_pad = np.zeros((NCORES, BLK * 128), np.int64)
    dhi_pad = np.zeros((NCORES, BLK * 128), np.int64)
    dlo_pad[:, :SH] = dlo_p
    dhi_pad[:, :SH] = dhi_p

    def _capcost(b0c, nbc):
        """Optimal hybrid slot cost (and tile count) for [b0c, b0c+nbc)."""
        tot = 0
        tiles = 0
        for dmat, kmax_arr in ((dlo_pad, blk_klo), (dhi_pad, blk_khi)):
            dgrp = dmat[:, b0c * 128:(b0c + nbc) * 128].reshape(
                NCORES, nbc, 128)
            kmax = int(kmax_arr[b0c:b0c + nbc].max())
            best = None
            for K in range(1, kmax + 1):
                ov = np.maximum(0, dgrp - K).sum(axis=2)
                ovt_b = ((ov + 127) // 128).max(axis=0)
                if int(ovt_b.max()) > 3:
                    continue
                cost = nbc * 128 * K + int(1.2 * 128 * ovt_b.sum())
                if best is None or cost < best[0]:
                    best = (cost, K, ovt_b)
            if best:
                tot += best[0]
                tiles += nbc * best[1] + int(best[2].sum())
            else:
                tot += nbc * 128 * kmax
                tiles += nbc * kmax
        return tot, tiles

    # adaptive groups on the CAPPED cost: overflow absorbs degree spread, so
    # groups can grow (fewer gather instructions) with little extra padding
    blk_cost = [_capcost(b, 1)[0] for b in range(BLK)]
    tgroups = []                                    # (b0, nb, KLO, KHI)
    b0 = 0
    while b0 < BLK:
        nb = 1
        while b0 + nb < BLK and nb < G1:
            c, tl_ = _capcost(b0, nb + 1)
            if c > 1.06 * sum(blk_cost[b0:b0 + nb + 1]) or tl_ > 240:
                break
            nb += 1
        tgroups.append((b0, nb, int(blk_klo[b0:b0 + nb].max()),
                        int(blk_khi[b0:b0 + nb].max())))
        b0 += nb
    # ---- hybrid: cap tree K per group; overflow edges go to a small
    # position-free S-matmul zone riding the same gathers
    OVT = np.zeros((BLK, 2), np.int64)              # ov tiles per (block, half)
    tgroups2 = []                                   # (b0, nb, cap_lo, cap_hi)
    for (b0g, nb, klo, khi) in tgroups:
        caps = []
        for h, (dmat, kmax) in enumerate(((dlo_pad, klo), (dhi_pad, khi))):
            dgrp = dmat[:, b0g * 128:(b0g + nb) * 128].reshape(NCORES, nb, 128)
            best = None
            for K in range(1, kmax + 1):
                ov = np.maximum(0, dgrp - K).sum(axis=2)          # [8, nb]
                ovt_b = ((ov + 127) // 128).max(axis=0)           # [nb]
                if int(ovt_b.max()) > 3:
                    continue   # bound per-block S-build size
                # overflow tiles cost ~1.5x a tree tile (adds DVE S-build
                # and PE matmul on top of the same DMA descriptors)
                cost = nb * 128 * K + int(1.2 * 128 * ovt_b.sum())
                if best is None or cost < best[0]:
                    best = (cost, K, ovt_b)
            caps.append(best)
        (_, cap_lo, ovt_lo), (_, cap_hi, ovt_hi) = caps
        tgroups2.append((b0g, nb, int(cap_lo), int(cap_hi)))
        OVT[b0g:b0g + nb, 0] = ovt_lo
        OVT[b0g:b0g + nb, 1] = ovt_hi
    tgroups = tgroups2
    toff1 = np.zeros(len(tgroups) + 1, np.int64)
    ovbase = np.zeros((BLK, 2), np.int64)           # global ov tile base
    for g, (b0g, nb, klo, khi) in enumerate(tgroups):
        ovlo = int(OVT[b0g:b0g + nb, 0].sum())
        ovhi = int(OVT[b0g:b0g + nb, 1].sum())
        run = toff1[g] + nb * klo
        for b in range(b0g, b0g + nb):
            ovbase[b, 0] = run
            run += OVT[b, 0]
        run = toff1[g] + nb * klo + ovlo + nb * khi
        for b in range(b0g, b0g + nb):
            ovbase[b, 1] = run
            run += OVT[b, 1]
        toff1[g + 1] = toff1[g] + nb * (klo + khi) + ovlo + ovhi
    TOT1 = int(toff1[-1])                           # tree + overflow tiles
    # per-block lookup arrays for the edge fill
    blk_group = np.zeros(BLK, np.int64)
    for g, (b0g, nb, klo, khi) in enumerate(tgroups):
        blk_group[b0g:b0g + nb] = g
    arr_b0 = np.array([t[0] for t in tgroups])
    arr_nb = np.array([t[1] for t in tgroups])
    arr_klo = np.array([t[2] for t in tgroups])
    arr_khi = np.array([t[3] for t in tgroups])
    arr_ovlo = np.array([int(OVT[t[0]:t[0] + t[1], 0].sum()) for t in tgroups])

    # per-(target, half) running k via stable sort
    key = tpos * 2 + ehalf + (tcore * SH * 2)
    order = np.argsort(key, kind="stable")
    ks = np.empty(E + N, np.int64)
    sk = key[order]
    run_start = np.r_[0, np.nonzero(np.diff(sk))[0] + 1]
    run_id = np.zeros(E + N, np.int64)
    run_id[run_start[1:]] = 1
    run_id = np.cumsum(run_id)
    ks[order] = np.arange(E + N) - run_start[run_id]

    gidx1, crels1 = [], []
    for k in range(NCORES):
        m = tcore == k
        b = tblk[m]
        g = blk_group[b]
        bi = b - arr_b0[g]
        capv = np.where(ehalf[m] == 0, arr_klo[g], arr_khi[g])
        tree = ks[m] < capv
        tile_idx = np.where(
            ehalf[m] == 0,
            toff1[g] + bi * arr_klo[g] + ks[m],
            toff1[g] + arr_nb[g] * arr_klo[g] + arr_ovlo[g]
            + bi * arr_khi[g] + ks[m],
        )
        pos_tree = tile_idx[tree] * 128 + trel[m][tree]
        # overflow: pack densely per (block, half)
        bo = b[~tree]
        ho = ehalf[m][~tree]
        keyo = bo * 2 + ho
        ordo = np.argsort(keyo, kind="stable")
        sko = keyo[ordo]
        rs = np.r_[0, np.nonzero(np.diff(sko))[0] + 1]
        rid = np.zeros(len(sko), np.int64)
        rid[rs[1:]] = 1
        rid = np.cumsum(rid)
        seq = np.empty(len(sko), np.int64)
        seq[ordo] = np.arange(len(sko)) - rs[rid]
        pos_ov = ovbase[bo, ho] * 128 + seq
        flat = np.full(TOT1 * 128, ZROW, np.int64)
        flat[pos_tree] = gloc[m][tree]
        flat[pos_ov] = gloc[m][~tree]
        cfl = np.full(TOT1 * 128, PADCREL, np.float64)
        cfl[pos_ov] = trel[m][~tree]
        gidx1.append(_wrap16(flat))
        crels1.append(np.ascontiguousarray(
            cfl.reshape(TOT1, 128).T.astype(bf16)))

    # ---- S-matmul stream (A2): groups of G2 blocks, [lo tiles | hi tiles]
    key2 = ((tcore * BLK) + tblk) * 2 + ehalf
    cnt = np.bincount(key2, minlength=NCORES * BLK * 2).reshape(NCORES, BLK, 2)
    TL = np.maximum(1, ((cnt + 127) // 128).max(axis=0))        # [BLK, 2]
    # groups sized by tile budget (SBUF for S + msg slabs)
    S_BUDGET = 72
    sgroups = []                                    # (b0, nb)
    b0 = 0
    while b0 < BLK:
        nb = 1
        tot = int(TL[b0].sum())
        while b0 + nb < BLK and nb < 8:
            nxt = int(TL[b0 + nb].sum())
            if tot + nxt > S_BUDGET:
                break
            tot += nxt
            nb += 1
        sgroups.append((b0, nb))
        b0 += nb
    toff2 = np.zeros(len(sgroups) + 1, np.int64)
    LOg = np.zeros(len(sgroups), np.int64)
    HIg = np.zeros(len(sgroups), np.int64)
    tile_of_bh = np.zeros((BLK, 2), np.int64)       # tile offset of (b, h)
    for g, (b0g, nb) in enumerate(sgroups):
        bs = range(b0g, b0g + nb)
        LOg[g] = sum(int(TL[b, 0]) for b in bs)
        HIg[g] = sum(int(TL[b, 1]) for b in bs)
        o = toff2[g]
        for b in bs:
            tile_of_bh[b, 0] = o
            o += TL[b, 0]
        for b in bs:
            tile_of_bh[b, 1] = o
            o += TL[b, 1]
        toff2[g + 1] = toff2[g] + LOg[g] + HIg[g]
    TOT2 = int(toff2[-1])

    order2 = np.lexsort((gloc, key2))
    key2s = key2[order2]
    gloc2 = gloc[order2]
    crel2 = trel[order2]
    starts = np.zeros(NCORES * BLK * 2 + 1, np.int64)
    np.cumsum(cnt.reshape(-1), out=starts[1:])

    gidx2, crels2 = [], []
    for k in range(NCORES):
        flat = np.full(TOT2 * 128, ZROW, np.int64)
        cfl = np.full(TOT2 * 128, PADCREL, np.float64)
        for b in range(BLK):
            for h in (0, 1):
                s = starts[(k * BLK + b) * 2 + h]
                e = starts[(k * BLK + b) * 2 + h + 1]
                n = int(e - s)
                base = tile_of_bh[b, h] * 128
                flat[base:base + n] = gloc2[s:e]
                cfl[base:base + n] = crel2[s:e]
        gidx2.append(_wrap16(flat))
        crels2.append(np.ascontiguousarray(
            cfl.reshape(TOT2, 128).T.astype(bf16)))

    meta = dict(tgroups=tgroups, toff1=toff1, TOT1=TOT1, OVT=OVT,
                ovbase=ovbase,
                TL=TL, sgroups=sgroups, LOg=LOg, HIg=HIg, toff2=toff2,
                TOT2=TOT2, tile_of_bh=tile_of_bh,
                SBW=max(int(TL.max()), int(OVT.sum(axis=1).max()) + 2))
    return deg, dinv, perms, gidx1, crels1, gidx2, crels2, meta


def _build_program(meta, sim_collectives=False):
    import concourse.bacc as bacc
    import concourse.mybir as mybir
    import concourse.tile as tile
    from concourse import ap_utils
    from concourse.bass import MemorySpace
    from concourse._compat import exact_div

    dt = mybir.dt
    nc = bacc.Bacc("TRN2", target_bir_lowering=False, debug=False,
                   num_devices=1 if sim_collectives else NCORES)

    def collective(ins, outs):
        if sim_collectives:
            o = outs[0]
            if len(o.shape) == 3:
                nc.sync.dma_start(out=o[0, :, :], in_=ins[0][:])
            else:
                nc.sync.dma_start(out=o[0:ins[0].shape[0], :], in_=ins[0][:])
        else:
            nc.gpsimd.collective_compute(
                "AllGather", mybir.AluOpType.bypass,
                replica_groups=[list(range(NCORES))], ins=ins, outs=outs)

    def gather_raw(out_ap, in_ap, idxs_ap, num_idxs, elem_size, elem_step):
        gp = nc.gpsimd
        assert idxs_ap.dtype == mybir.dt.int16
        assert in_ap.space == MemorySpace.DRAM
        assert ap_utils.ap_is_contiguous(out_ap.ap[1:])
        assert ap_utils.ap_is_contiguous(idxs_ap.ap[1:])
        assert in_ap.ap[-1][1] == out_ap.ap[-1][1] == elem_size
        assert in_ap.ap[0][0] == elem_step
        stride_bytes = elem_step * mybir.dt.size(in_ap.dtype)
        stride_bytes_256 = exact_div(stride_bytes, 256)
        _in_ap = gp.lower_ap_dma(in_ap, for_custom_bir_dma=True)
        return gp.add_instruction(
            mybir.InstDMAGatherAnt(
                name=gp.bass.get_next_instruction_name(),
                ins=[*_in_ap, gp.lower_ap(idxs_ap),
                     gp.lower_val_access(gp.to_reg(num_idxs))],
                outs=[gp.lower_ap(out_ap)],
                transpose=False, num_idxs=num_idxs, elem_size=elem_size,
                stride_bytes_256=stride_bytes_256, gen_mode=0,
                single_packet=False, queue_num=0,
            ))

    tgroups, toff1, TOT1 = meta["tgroups"], meta["toff1"], meta["TOT1"]
    OVT, ovbase = meta["OVT"], meta["ovbase"]
    TL, sgroups, LOg, HIg = meta["TL"], meta["sgroups"], meta["LOg"], meta["HIg"]
    toff2, TOT2 = meta["toff2"], meta["TOT2"]
    tile_of_bh = meta["tile_of_bh"]

    # ---- dram tensors
    xpad = nc.dram_tensor("xpad", [NCORES * CH, 128], dt.bfloat16,
                          kind="ExternalInput")
    dloc_in = nc.dram_tensor("dloc_in", [128, BLK], dt.float32,
                             kind="ExternalInput")
    W1s = nc.dram_tensor("W1s", [16, H], dt.bfloat16, kind="ExternalInput")
    W2T = nc.dram_tensor("W2T", [H, H], dt.bfloat16, kind="ExternalInput")
    b2s = nc.dram_tensor("b2s", [16, H], dt.bfloat16, kind="ExternalInput")
    W3T16 = nc.dram_tensor("W3T16", [H, 16], dt.bfloat16, kind="ExternalInput")
    b3r = nc.dram_tensor("b3r", [1, 16], dt.bfloat16, kind="ExternalInput")
    onesr = nc.dram_tensor("onesr", [1, H], dt.bfloat16, kind="ExternalInput")
    Rfl = nc.dram_tensor("Rfl", [128, C * C], dt.float32, kind="ExternalInput")
    SBW = meta["SBW"]
    iota_in = nc.dram_tensor("iota_in", [128, 128 * SBW], dt.bfloat16,
                             kind="ExternalInput")
    idb_in = nc.dram_tensor("idb_in", [128, 128], dt.bfloat16,
                            kind="ExternalInput")
    gi1 = nc.dram_tensor("gi1", [128, TOT1 * 8], dt.int16, kind="ExternalInput")
    cr1 = nc.dram_tensor("cr1", [128, TOT1], dt.bfloat16, kind="ExternalInput")
    gi2 = nc.dram_tensor("gi2", [128, TOT2 * 8], dt.int16, kind="ExternalInput")
    cr2 = nc.dram_tensor("cr2", [128, TOT2], dt.bfloat16, kind="ExternalInput")
    out = nc.dram_tensor("out", [SH, C], dt.float32, kind="ExternalOutput")

    gin2 = nc.dram_tensor("gin2", [CH, 256], dt.float8e4)
    gout2 = nc.dram_tensor("gout2", [NCORES * CH, 256], dt.float8e4,
                           addr_space="Shared")
    gin3 = nc.dram_tensor("gin3", [CH, 16], dt.bfloat16)
    gout3 = nc.dram_tensor("gout3", [NCORES * CH, 16], dt.bfloat16,
                           addr_space="Shared")
    xpad3 = nc.dram_tensor("xpad3", [NCORES * CH, 128], dt.bfloat16)

    with tile.TileContext(nc) as tc:
        with (
            tc.tile_pool(name="const", bufs=1) as cpool,
            tc.tile_pool(name="cidx", bufs=6) as ipool,
            tc.tile_pool(name="slab", bufs=3) as slpool,
            tc.tile_pool(name="sb", bufs=3) as spool,
            tc.tile_pool(name="work", bufs=4) as wpool,
            tc.tile_pool(name="gt", bufs=4) as gpool,
            tc.tile_pool(name="psa", bufs=3, space="PSUM") as ppa,
            tc.tile_pool(name="pst", bufs=2, space="PSUM") as ppt,
            tc.tile_pool(name="psf", bufs=2, space="PSUM") as ppf,
            tc.tile_pool(name="pss", bufs=1, space="PSUM") as pps,
        ):
            # ---- consts
            w1_t = cpool.tile([16, H], dt.bfloat16)
            nc.sync.dma_start(out=w1_t[:], in_=W1s[:])
            w2_t = cpool.tile([H, H], dt.bfloat16)
            nc.sync.dma_start(out=w2_t[:], in_=W2T[:])
            b2_t = cpool.tile([16, H], dt.bfloat16)
            nc.sync.dma_start(out=b2_t[:], in_=b2s[:])
            w3_t = cpool.tile([H, 16], dt.bfloat16)
            nc.sync.dma_start(out=w3_t[:], in_=W3T16[:])
            b3_t = cpool.tile([1, 16], dt.bfloat16)
            nc.sync.dma_start(out=b3_t[:], in_=b3r[:])
            on_t = cpool.tile([1, H], dt.bfloat16)
            nc.sync.dma_start(out=on_t[:], in_=onesr[:])
            r_t = cpool.tile([128, C * C], dt.float32)
            nc.sync.dma_start(out=r_t[:], in_=Rfl[:])
            io_t = cpool.tile([128, 128, SBW], dt.bfloat16)
            nc.sync.dma_start(
                out=io_t[:], in_=iota_in[:].rearrange("p (c j) -> p c j", j=SBW))
            id_t = cpool.tile([128, 128], dt.bfloat16)
            nc.sync.dma_start(out=id_t[:], in_=idb_in[:])
            dl_t = cpool.tile([128, BLK], dt.float32)
            nc.sync.dma_start(out=dl_t[:], in_=dloc_in[:])
            # transposed L1 aggregates, kept for transforms + bias rows
            z1trs = cpool.tile([16, CH], dt.bfloat16)
            cr1_t = cpool.tile([128, TOT1], dt.bfloat16)
            nc.scalar.dma_start(out=cr1_t[:], in_=cr1[:])



            def tree_groups(table, width_slab, z_consumer):
                """A1/A3: gather + tree-reduce per group (+ overflow S-mm);
                slab zones: [tree-lo | ov-lo | tree-hi | ov-hi]."""
                for g, (b0g, nb, klo, khi) in enumerate(tgroups):
                    ovlo = int(OVT[b0g:b0g + nb, 0].sum())
                    ovhi = int(OVT[b0g:b0g + nb, 1].sum())
                    ntiles = nb * (klo + khi) + ovlo + ovhi
                    lo_tiles = nb * klo + ovlo
                    o1 = int(toff1[g]) * 8
                    gi_t = ipool.tile([128, ntiles * 8], dt.int16, tag="gi1c")
                    nc.scalar.dma_start(out=gi_t[:],
                                        in_=gi1[:, o1:o1 + ntiles * 8])
                    slab = slpool.tile([128, width_slab, 16], dt.bfloat16,
                                       tag="slab1")
                    nlo = nb * klo
                    gather_raw(slab[:, 0:lo_tiles, :], table[0:HALF, 0:16],
                               gi_t[:, 0:lo_tiles * 8], lo_tiles * 128, 16, 128)
                    gather_raw(slab[:, lo_tiles:ntiles, :],
                               table[HALF:2 * HALF, 0:16],
                               gi_t[:, lo_tiles * 8:ntiles * 8],
                               (ntiles - lo_tiles) * 128, 16, 128)

                    def reduce_zone(zone, k):
                        # zone: [128, nb, k, 16] view; halve in place
                        while k > 1:
                            h = k // 2
                            nc.vector.tensor_tensor(
                                out=zone[:, :, 0:h, :], in0=zone[:, :, 0:h, :],
                                in1=zone[:, :, h:2 * h, :],
                                op=mybir.AluOpType.add)
                            if k % 2:
                                nc.vector.tensor_tensor(
                                    out=zone[:, :, 0:1, :],
                                    in0=zone[:, :, 0:1, :],
                                    in1=zone[:, :, k - 1:k, :],
                                    op=mybir.AluOpType.add)
                            k = h
                        return zone[:, :, 0, :]

                    zl = reduce_zone(
                        slab[:, 0:nlo, :].rearrange("p (b k) e -> p b k e",
                                                    b=nb), klo)
                    zh = reduce_zone(
                        slab[:, lo_tiles:lo_tiles + nb * khi, :].rearrange(
                            "p (b k) e -> p b k e", b=nb), khi)
                    zg = gpool.tile([128, nb, 16], dt.bfloat16, tag="zg")
                    nc.vector.tensor_tensor(out=zg[:], in0=zl, in1=zh,
                                            op=mybir.AluOpType.add)
                    # overflow S-matmuls per block (if any ov tiles)
                    for bi in range(nb):
                        b = b0g + bi
                        novl, novh = int(OVT[b, 0]), int(OVT[b, 1])
                        if novl + novh == 0:
                            continue
                        runs = []
                        if novl:
                            runs.append((int(ovbase[b, 0]), novl))
                        if novh:
                            runs.append((int(ovbase[b, 1]), novh))
                        Sov = spool.tile([128, 128, OVMAX], dt.bfloat16,
                                         tag="Sov")
                        srel = 0
                        for (t0, n) in runs:
                            nc.vector.tensor_tensor(
                                out=Sov[:, :, srel:srel + n],
                                in0=cr1_t[:, t0:t0 + n].unsqueeze(1)
                                .broadcast_to([128, 128, n]),
                                in1=io_t[:, :, 0:n],
                                op=mybir.AluOpType.is_equal)
                            srel += n
                        aov = ppa.tile([128, 16], dt.float32, tag="agg")
                        srel = 0
                        tot = novl + novh
                        for (t0, n) in runs:
                            for j in range(n):
                                st = int(t0 - toff1[g]) + j
                                nc.tensor.matmul(
                                    aov[:, :], Sov[:, :, srel + j],
                                    slab[:, st, :],
                                    start=(srel + j == 0),
                                    stop=(srel + j == tot - 1))
                            srel += n
                        nc.vector.tensor_tensor(
                            out=zg[:, bi, :], in0=zg[:, bi, :],
                            in1=aov[:, :], op=mybir.AluOpType.add)
                    z_consumer(b0g, nb, zg)

            # ---- A1 + T1
            SLAB1W = max(
                t[1] * (t[2] + t[3]) + int(OVT[t[0]:t[0] + t[1]].sum())
                for t in tgroups)
            OVMAX = int(OVT.sum(axis=1).max())

            def t1_consumer(b0g, nb, zg):
                for bi in range(nb):
                    b = b0g + bi
                    tp = ppt.tile([16, 128], dt.bfloat16, tag="tp")
                    nc.tensor.transpose(tp[:, :], zg[:, bi, :], id_t[:])
                    nc.vector.tensor_copy(z1trs[:, b * 128:(b + 1) * 128],
                                          tp[:, :])
                    hp = ppf.tile([128, H], dt.float32, tag="tf")
                    nc.tensor.matmul(hp[:, :],
                                     z1trs[:, b * 128:(b + 1) * 128], w1_t[:],
                                     start=True, stop=True)
                    h1 = wpool.tile([128, H], dt.bfloat16, tag="h1")
                    nc.scalar.activation(h1[:], hp[:, :],
                                         mybir.ActivationFunctionType.Relu,
                                         scale=dl_t[:, b:b + 1])
                    g2t = gpool.tile([128, H], dt.float8e4, tag="g2t")
                    nc.scalar.activation(g2t[:], h1[:],
                                         mybir.ActivationFunctionType.Copy,
                                         scale=dl_t[:, b:b + 1])
                    nc.sync.dma_start(out=gin2[b * 128:(b + 1) * 128, 0:128],
                                      in_=g2t[:])

            tree_groups(xpad, SLAB1W, t1_consumer)

            collective([gin2[:, :]], [gout2[:, :]])

            # ---- A2 + T2/T3
            for g, (b0, nb) in enumerate(sgroups):
                lo, hi = int(LOg[g]), int(HIg[g])
                tg = lo + hi
                o = int(toff2[g])
                gi_t = ipool.tile([128, tg * 8], dt.int16, tag="gi2c")
                nc.scalar.dma_start(out=gi_t[:], in_=gi2[:, o * 8:(o + tg) * 8])
                cr_t = ipool.tile([128, tg], dt.bfloat16, tag="cr2c")
                nc.scalar.dma_start(out=cr_t[:], in_=cr2[:, o:o + tg])
                slab8 = slpool.tile([128, tg, 128], dt.float8e4, tag="slab8")
                gather_raw(slab8[:, 0:lo, :], gout2[0:HALF, 0:128],
                           gi_t[:, 0:lo * 8], lo * 128, 128, 256)
                gather_raw(slab8[:, lo:tg, :], gout2[HALF:2 * HALF, 0:128],
                           gi_t[:, lo * 8:tg * 8], hi * 128, 128, 256)
                S = spool.tile([128, 128, tg], dt.bfloat16, tag="S")
                for bi in range(nb):
                    b = b0 + bi
                    tiles = (list(range(int(tile_of_bh[b, 0]) - o,
                                        int(tile_of_bh[b, 0]) - o + int(TL[b, 0])))
                             + list(range(int(tile_of_bh[b, 1]) - o,
                                          int(tile_of_bh[b, 1]) - o + int(TL[b, 1]))))
                    for t0, t1 in ((tiles[0], tiles[0] + int(TL[b, 0])),
                                   (tiles[int(TL[b, 0])],
                                    tiles[int(TL[b, 0])] + int(TL[b, 1]))):
                        nc.vector.tensor_tensor(
                            out=S[:, :, t0:t1],
                            in0=cr_t[:, t0:t1].unsqueeze(1).broadcast_to(
                                [128, 128, t1 - t0]),
                            in1=io_t[:, :, 0:t1 - t0],
                            op=mybir.AluOpType.is_equal)
                    acc = ppa.tile([128, H], dt.float32, tag="agg")
                    for idx, j in enumerate(tiles):
                        nc.tensor.matmul(acc[:, :], S[:, :, j], slab8[:, j, :],
                                         start=(idx == 0),
                                         stop=(idx == len(tiles) - 1))
                    z2sb = wpool.tile([128, H], dt.bfloat16, tag="z2sb")
                    nc.vector.tensor_copy(z2sb[:], acc[:, :])
                    tp2 = ppt.tile([128, H], dt.bfloat16, tag="tp")
                    nc.tensor.transpose(tp2[:, :], z2sb[:], id_t[:])
                    z2tr = wpool.tile([128, H], dt.bfloat16, tag="z2tr")
                    nc.vector.tensor_copy(z2tr[:], tp2[:, :])
                    hp2 = ppf.tile([128, H], dt.float32, tag="tf")
                    nc.tensor.matmul(hp2[:, :], z2tr[:], w2_t[:],
                                     start=True, stop=False)
                    nc.tensor.matmul(hp2[:, :],
                                     z1trs[:, b * 128:(b + 1) * 128],
                                     b2_t[:], start=False, stop=True)
                    h2 = wpool.tile([128, H], dt.bfloat16, tag="h2")
                    nc.scalar.activation(h2[:], hp2[:, :],
                                         mybir.ActivationFunctionType.Relu,
                                         scale=dl_t[:, b:b + 1])
                    tp3 = ppt.tile([128, H], dt.bfloat16, tag="tp")
                    nc.tensor.transpose(tp3[:, :], h2[:], id_t[:])
                    h2tr = wpool.tile([128, H], dt.bfloat16, tag="h2tr")
                    nc.vector.tensor_copy(h2tr[:], tp3[:, :])
                    gp3 = pps.tile([128, 16], dt.float32, tag="sm")
                    nc.tensor.matmul(gp3[:, :], h2tr[:], w3_t[:],
                                     start=True, stop=False)
                    nc.tensor.matmul(gp3[:, :], on_t[:, :], b3_t[:],
                                     start=False, stop=True)
                    g3t = gpool.tile([128, 16], dt.bfloat16, tag="g3t")
                    nc.scalar.activation(g3t[:], gp3[:, :],
                                         mybir.ActivationFunctionType.Copy,
                                         scale=dl_t[:, b:b + 1])
                    nc.sync.dma_start(out=gin3[b * 128:(b + 1) * 128, :],
                                      in_=g3t[:])

            collective([gin3[:, :]], [gout3[:, :]])
            nc.sync.dma_start(out=xpad3[0:HALF, 0:16], in_=gout3[0:HALF, :])
            nc.sync.dma_start(out=xpad3[HALF:2 * HALF, 0:16],
                              in_=gout3[HALF:2 * HALF, :])

            # ---- A3 + sigmoid + R-max
            def t3_consumer(b0g, nb, zg):
                for bi in range(nb):
                    b = b0g + bi
                    h3 = wpool.tile([128, 16], dt.float32, tag="h3")
                    nc.scalar.activation(h3[:], zg[:, bi, :],
                                         mybir.ActivationFunctionType.Sigmoid,
                                         scale=dl_t[:, b:b + 1])
                    tmp = wpool.tile([128, C, C], dt.float32, tag="tmp")
                    nc.vector.tensor_tensor(
                        out=tmp[:],
                        in0=h3[:, 0:C].unsqueeze(1).broadcast_to([128, C, C]),
                        in1=r_t[:].rearrange("p (a b) -> p a b", a=C),
                        op=mybir.AluOpType.mult)
                    o13 = gpool.tile([128, C], dt.float32, tag="o13")
                    nc.vector.tensor_reduce(o13[:], tmp[:],
                                            axis=mybir.AxisListType.X,
                                            op=mybir.AluOpType.max)
                    rows = 128 if b < BLK - 1 else LASTB
                    nc.sync.dma_start(out=out[b * 128:b * 128 + rows, :],
                                      in_=o13[0:rows, :])

            tree_groups(xpad3, SLAB1W, t3_consumer)

    nc.compile()
    return nc


def kernel(x, edge_index, R, W1, b1, W2, b2, W3, b3, **_):
    global LAST_RESULTS
    import concourse.mybir  # noqa: F401
    from concourse.bass_utils import run_bass_kernel_spmd

    x = np.asarray(x, np.float32)
    edge_index = np.asarray(edge_index, np.int32)
    deg, dinv, perms, gidx1, crels1, gidx2, crels2, meta = _prep_edges(edge_index)

    nc = _build_program(meta)

    # global permuted-padded L1 table, built on host: [dinv*x, dinv, 0...]
    dpad = np.zeros(NCORES * CH, np.float32)
    xtp = np.zeros([NCORES * CH, 128], bf16)
    for k in range(NCORES):
        dpad[k * CH:k * CH + SH] = dinv[k * SH + perms[k]]
        xs = x[k * SH + perms[k]] * dpad[k * CH:k * CH + SH, None]
        xtp[k * CH:k * CH + SH, 0:DIN] = xs.astype(bf16)
        xtp[k * CH:k * CH + SH, DIN] = dpad[k * CH:k * CH + SH].astype(bf16)

    W1stack = np.zeros([16, H], np.float32)
    W1stack[0:DIN] = np.asarray(W1, np.float32).T
    W1stack[DIN] = np.asarray(b1, np.float32)
    b2stack = np.zeros([16, H], np.float32)
    b2stack[DIN] = np.asarray(b2, np.float32)
    W3p = np.zeros([H, 16], np.float32)
    W3p[:, 0:C] = np.asarray(W3, np.float32).T
    b3p = np.zeros([1, 16], np.float32)
    b3p[0, 0:C] = np.asarray(b3, np.float32)
    Rfl = np.tile(np.asarray(R, np.float32).reshape(1, C * C), (128, 1))
    SBW = meta["SBW"]
    iota = np.tile(
        np.repeat(np.arange(128, dtype=np.float32), SBW).astype(bf16)[None, :],
        (128, 1))
    idb = np.eye(128, dtype=np.float32).astype(bf16)

    in_maps = []
    for k in range(NCORES):
        dloc = np.ascontiguousarray(
            dpad[k * CH:(k + 1) * CH].reshape(BLK, 128).T)
        in_maps.append({
            "xpad": xtp, "dloc_in": dloc,
            "W1s": W1stack.astype(bf16),
            "W2T": np.ascontiguousarray(np.asarray(W2, np.float32).T).astype(bf16),
            "b2s": b2stack.astype(bf16),
            "W3T16": W3p.astype(bf16), "b3r": b3p.astype(bf16),
            "onesr": np.ones([1, H], np.float32).astype(bf16),
            "Rfl": Rfl, "iota_in": iota, "idb_in": idb,
            "gi1": gidx1[k], "cr1": crels1[k],
            "gi2": gidx2[k], "cr2": crels2[k],
        })

    trace = os.environ.get("GNN_TRACE") == "1"
    res = run_bass_kernel_spmd(nc, in_maps, core_ids=list(range(NCORES)),
                               trace=trace)
    LAST_RESULTS = res

    reps = int(os.environ.get("GNN_BENCH", "0"))
    if reps > 0:
        _bench(nc, in_maps, reps)

    outp = np.empty([N, C], np.float32)
    for k in range(NCORES):
        outp[k * SH + perms[k]] = res.results[k]["out"]
    return outp


BENCH_TIMES = None
BENCH_PIPELINED_NS = None


def _bench(nc, in_maps, reps):
    """Time repeated executions of the already-built program through a single
    jit instance (NEFF compile amortized away; inputs device_put once)."""
    global BENCH_TIMES
    import time
    import jax
    from jax.sharding import Mesh, PartitionSpec, NamedSharding
    from jax.experimental.shard_map import shard_map
    import concourse.mybir as mybir
    from concourse.bass2jax import (_bass_exec_p, partition_id_tensor,
                                    install_neuronx_cc_hook)

    install_neuronx_cc_hook()
    in_names, out_names, out_avals, zero_outs = [], [], [], []
    pname = nc.partition_id_tensor.name if nc.partition_id_tensor else None
    for alloc in nc.m.functions[0].allocations:
        if not isinstance(alloc, mybir.MemoryLocationSet):
            continue
        name = alloc.memorylocations[0].name
        if alloc.kind == "ExternalInput":
            if name != pname:
                in_names.append(name)
        elif alloc.kind == "ExternalOutput":
            out_names.append(name)
            shape = tuple(alloc.tensor_shape)
            dtype = mybir.dt.np(alloc.dtype)
            out_avals.append(jax.core.ShapedArray(shape, dtype))
            zero_outs.append(np.zeros(shape, dtype))
    n_params = len(in_names)
    all_names = in_names + out_names + ([pname] if pname else [])

    def _body(*args):
        ops = list(args)
        if pname:
            ops.append(partition_id_tensor())
        return tuple(_bass_exec_p.bind(
            *ops, out_avals=tuple(out_avals), in_names=tuple(all_names),
            out_names=tuple(out_names), lowering_input_output_aliases=(),
            sim_require_finite=True, sim_require_nnan=True, nc=nc))

    devices = jax.devices()[:NCORES]
    mesh = Mesh(np.asarray(devices), ("core",))
    nouts = len(out_names)
    sharded = jax.jit(
        shard_map(_body, mesh=mesh,
                  in_specs=(PartitionSpec("core"),) * (n_params + nouts),
                  out_specs=(PartitionSpec("core"),) * nouts, check_rep=False),
        donate_argnums=tuple(range(n_params, n_params + nouts)),
        keep_unused=True)
    sh = NamedSharding(mesh, PartitionSpec("core"))
    dev_in = [jax.device_put(
        np.concatenate([np.asarray(in_maps[c][nm]) for c in range(NCORES)], axis=0), sh)
        for nm in in_names]
    times = []
    for i in range(reps + 1):
        zs = [jax.device_put(
            np.zeros((NCORES * z.shape[0], *z.shape[1:]), z.dtype), sh)
            for z in zero_outs]
        t0 = time.perf_counter()
        outs = sharded(*dev_in, *zs)
        jax.block_until_ready(outs)
        times.append(time.perf_counter() - t0)
    BENCH_TIMES = times
    print("bench wall times (s):", " ".join(f"{t:.4f}" for t in times))
    print(f"bench min/median after warmup: {min(times[1:]):.4f} / "
          f"{sorted(times[1:])[len(times[1:]) // 2]:.4f}")

    NPIPE = 6
    zss = [[jax.device_put(
        np.zeros((NCORES * z.shape[0], *z.shape[1:]), z.dtype), sh)
        for z in zero_outs] for _ in range(NPIPE)]
    t0 = time.perf_counter()
    outs = None
    for i in range(NPIPE):
        outs = sharded(*dev_in, *zss[i])
    jax.block_until_ready(outs)
    tp = (time.perf_counter() - t0) / NPIPE
    global BENCH_PIPELINED_NS
    BENCH_PIPELINED_NS = int(tp * 1e9)
    print(f"bench pipelined per-exec: {tp * 1e3:.3f} ms "
          f"({tp * 1e9:.0f} ns upper bound)")
